# revision 1
# baseline (speedup 1.0000x reference)
"""Trainium2 Bass kernel for nn_AttentionBlock (B=16, C=512, H=W=32, 8 heads).

Sharding: data-parallel over batch across 8 NeuronCores (2 batch elems/core).
No collectives: each core runs the same NEFF on its own batch slice.

Key structure (v2 — overlap-oriented rewrite):
  - All matmuls bf16, N=512, ~219ns/MM measured back-to-back (LDWEIGHTS hides).
  - QKV weights pair-packed so q/k of a head pair land on psum partition
    halves in one [128,1024] chain -> single full-width psum->SBUF copy.
  - Attention: per pair, per j-tile: scores (K=64) -> ScalarE exp (the pacer,
    ~1.1us per [128,1024] ACT); P@V uses the ones-column trick (M=65) for the
    softmax denominator, two sc-passes so its PSUM footprint is 2 banks.
  - Normalization: reciprocal straight from PSUM row 64, GpSimd broadcast,
    DVE multiply into oT.
  - Residual added from the bf16 x copy during the proj psum->SBUF move
    (no DRAM->DRAM precopy, no accumulating DMA).
  - Engine queues are FIFO in emission order, so elem1's QKV chains and
    elem0's proj chains are *emitted inside* elem0/elem1's attention pair
    loops (hooks) to fill PE slack under the ScalarE-paced softmax.
  PSUM budget: scores 2x[128,1024] (4 banks) + PV 2x[65,512] (2 banks)
  + one [128,1024] chain lane (2 banks) = 8 banks.
"""

import numpy as np

import concourse.bacc as bacc
import concourse.bass as bass
import concourse.mybir as mybir
import concourse.tile as tile

F32 = mybir.dt.float32
BF16 = mybir.dt.bfloat16

B, C, HW, NH, DK = 16, 512, 1024, 8, 64
NCORES = 8
BPC = B // NCORES          # batch elems per core
P = 128
KT = C // P                # 4 contraction tiles over C
NPAIR = NH // 2            # 4 head pairs
SC = HW // 512             # 2 s-chunks of 512
ST = HW // P               # 8 s-tiles of 128 (j tiles)
SCALE = DK ** -0.5

MM_DTYPE = "bf16"  # kept for test.py compat; kernel always runs bf16 matmuls


def build_program(with_bias: bool, mm_dtype: str = MM_DTYPE):
    nc = bacc.Bacc(None, target_bir_lowering=False, debug=False)

    x_d = nc.dram_tensor("x", [BPC, C, HW], F32, kind="ExternalInput")
    wqkv_d = nc.dram_tensor("w_qkv", [C, 3 * C], F32, kind="ExternalInput")
    wproj_d = nc.dram_tensor("w_proj", [C, C], F32, kind="ExternalInput")
    if with_bias:
        bqkv_d = nc.dram_tensor("b_qkv", [3 * C], F32, kind="ExternalInput")
        bproj_d = nc.dram_tensor("b_proj", [C], F32, kind="ExternalInput")
    out_d = nc.dram_tensor("out", [BPC, C, HW], BF16,
                           kind="ExternalOutput")

    with tile.TileContext(nc) as tc:
        with tc.tile_pool(name="consts", bufs=1) as consts:
            # Pair-packed q/k weights: [kt, pair, {q,k}, (m*64+t)] where the
            # 128 columns of (pair a, qk) are [w_{2a} | w_{2a+1}] head halves.
            wqk_r = consts.tile([P, KT, NPAIR, 2, P], BF16)
            wv_sb = consts.tile([P, KT, C], BF16)
            wproj_r = consts.tile([P, KT, C], BF16)
            warm_i = consts.tile([1, DK], F32)
            warm_o = consts.tile([1, DK], F32)
            # Zero-padded q: per (pair, m) the rhs is [128, S] with q_h on
            # partition half m and ZEROS on the other half, so score matmuls
            # run K=128 (no 64-row tiling mode switch, which drains the PE).
            qzp = [consts.tile([P, NPAIR, 2, HW], BF16, name=f"qzp{b}")
                   for b in range(BPC)]
            if with_bias:
                bq_sb = consts.tile([P, NPAIR], F32)
                bk_sb = consts.tile([P, NPAIR], F32)
                bv_sb = consts.tile([1, C], BF16)
                bp_sb = consts.tile([P, KT], F32)
                ones_f32 = consts.tile([1, P], F32)
                nc.vector.memset(ones_f32, 1.0)
                ones_sb = consts.tile([1, P], BF16)
                nc.vector.tensor_copy(out=ones_sb, in_=ones_f32)

            nc.vector.memset(warm_i, 1.0)
            # half-masks: qzp is written as q * mask so its zero half never
            # needs a bulk memset
            hmask = consts.tile([P, 2], F32)
            nc.vector.memset(hmask, 0.0)
            nc.vector.memset(hmask[0:DK, 0:1], 1.0)
            nc.vector.memset(hmask[DK:P, 1:2], 1.0)

            # Weight staging: all wqkv chunks issued up front on the
            # scalar HWDGE queue (parallel rings); x rides the sync queue;
            # wproj (not needed until proj(0)) goes to the slow gpsimd SWDGE.
            with tc.tile_pool(name="wstage", bufs=4) as wstagep:
                wsts = []
                for kt in range(KT):
                    wst = wstagep.tile([P, 3 * C], F32, tag="wq",
                                       name=f"wst{kt}", bufs=4)
                    nc.scalar.dma_start(
                        out=wst, in_=wqkv_d[kt * P : (kt + 1) * P, :]
                    )
                    wsts.append(wst)
                # Warm the ScalarE exp table set (after the DMA triggers so
                # it doesn't delay them; before any real ACT).
                nc.scalar.activation(
                    out=warm_o, in_=warm_i,
                    func=mybir.ActivationFunctionType.Exp, scale=1.0,
                )
                for kt in range(KT):
                    ws4 = wsts[kt].rearrange("p (h t) -> p h t", t=3 * DK)
                    for qk in range(2):
                        # pair a's 128 cols = heads (2a, 2a+1) side by side
                        nc.vector.tensor_copy(
                            out=wqk_r[:, kt, :, qk, :].rearrange(
                                "p a (m t) -> p a m t", m=2),
                            in_=ws4[:, :, qk * DK : (qk + 1) * DK].rearrange(
                                "p (a m) t -> p a m t", m=2),
                        )
                for kt in range(KT):
                    ws4 = wsts[kt].rearrange("p (h t) -> p h t", t=3 * DK)
                    nc.vector.tensor_copy(
                        out=wv_sb[:, kt, :].rearrange("p (h t) -> p h t", t=DK),
                        in_=ws4[:, :, 2 * DK :],
                    )
                if with_bias:
                    b3 = bqkv_d[:].rearrange("(h t) -> h t", t=3 * DK)
                    for m in range(2):
                        nc.sync.dma_start(
                            out=bq_sb[m * DK : (m + 1) * DK, :],
                            in_=b3[m::2, 0:DK].rearrange("a t -> t a"),
                        )
                        nc.sync.dma_start(
                            out=bk_sb[m * DK : (m + 1) * DK, :],
                            in_=b3[m::2, DK : 2 * DK].rearrange("a t -> t a"),
                        )
                    bv_st = wstagep.tile([1, C], F32, tag="bv")
                    nc.sync.dma_start(
                        out=bv_st,
                        in_=b3[:, 2 * DK :].rearrange("h t -> (h t)")[None, :],
                    )
                    nc.vector.tensor_copy(out=bv_sb, in_=bv_st)
                    nc.sync.dma_start(
                        out=bp_sb, in_=bproj_d[:].rearrange("(a p) -> p a", p=P)
                    )

            with (
                tc.tile_pool(name="xf", bufs=1) as xfp,
                tc.tile_pool(name="xr", bufs=2) as xrp,
                tc.tile_pool(name="qk", bufs=2) as qkp,
                tc.tile_pool(name="vp", bufs=2) as vpp,
                tc.tile_pool(name="pt", bufs=4) as ptp,
                tc.tile_pool(name="ot", bufs=2) as otp,
                tc.tile_pool(name="yt", bufs=3) as ytp,
                tc.tile_pool(name="rc", bufs=2) as rcp,
                tc.tile_pool(name="rcb", bufs=2) as rcbp,
                tc.tile_pool(name="psS", bufs=2, space="PSUM") as psS,
                tc.tile_pool(name="psQ", bufs=1, space="PSUM") as psQ,
                tc.tile_pool(name="psV", bufs=2, space="PSUM") as psV,
            ):
                x_r = [None, None]
                kT = [None, None]
                v_sb = [None, None]
                oT = [None, None]

                def emit_load(b):
                    """DMA x (f32, in 2 chunks) and cast to bf16 x_r."""
                    x_r[b] = xrp.tile([P, KT, HW], BF16, tag="xr",
                                      name=f"xr{b}")
                    xv = x_d[b].rearrange("(kt p) s -> p kt s", p=P)
                    for ch in range(2):
                        xc = xfp.tile([P, 2, HW], F32, tag="x",
                                      name=f"x{b}_{ch}")
                        nc.sync.dma_start(
                            out=xc, in_=xv[:, 2 * ch : 2 * ch + 2, :]
                        )
                        nc.vector.tensor_copy(
                            out=x_r[b][:, 2 * ch : 2 * ch + 2, :].rearrange(
                                "p k s -> p (k s)"),
                            in_=xc.rearrange("p k s -> p (k s)"),
                        )

                def alloc_attn_bufs(b):
                    kT[b] = qkp.tile([P, NPAIR, HW], BF16, tag="kT",
                                     name=f"kT{b}")
                    v_sb[b] = vpp.tile([P, ST, NH, DK + 1], BF16, tag="v",
                                       name=f"v{b}")
                    oT[b] = otp.tile([P, NPAIR, HW], BF16, tag="oT",
                                     name=f"oT{b}")
                    nc.vector.memset(
                        v_sb[b].rearrange("p st h t -> p (st h) t")[:, :, DK:],
                        1.0,
                    )

                def emit_chain_qk(b, a, qk, pool):
                    """One q-or-k chain for head pair a -> qkT[b][:,qk,a,:]."""
                    ps = pool.tile([P, HW], F32, tag=pool_tag[id(pool)],
                                   name=f"qk{b}_{a}_{qk}")
                    for kt in range(KT):
                        for sc in range(SC):
                            nc.tensor.matmul(
                                ps[:, sc * 512 : (sc + 1) * 512],
                                lhsT=wqk_r[:, kt, a, qk, :],
                                rhs=x_r[b][:, kt, sc * 512 : (sc + 1) * 512],
                                start=(kt == 0),
                                stop=(kt == KT - 1),
                            )
                    if qk == 0:
                        for m in range(2):
                            if with_bias:
                                nc.vector.tensor_scalar(
                                    out=qzp[b][:, a, m, :], in0=ps,
                                    scalar1=bq_sb[:, a : a + 1],
                                    scalar2=hmask[:, m : m + 1],
                                    op0=mybir.AluOpType.add,
                                    op1=mybir.AluOpType.mult,
                                )
                            else:
                                nc.vector.tensor_scalar(
                                    out=qzp[b][:, a, m, :], in0=ps,
                                    scalar1=hmask[:, m : m + 1],
                                    scalar2=None,
                                    op0=mybir.AluOpType.mult,
                                )
                    elif with_bias:
                        nc.vector.tensor_scalar(
                            out=kT[b][:, a, :], in0=ps,
                            scalar1=bk_sb[:, a : a + 1], scalar2=None,
                            op0=mybir.AluOpType.add,
                        )
                    else:
                        nc.vector.tensor_copy(out=kT[b][:, a, :], in_=ps)

                def emit_chain_v(b, mt2, pool):
                    """v rows for s-tiles 2*mt2, 2*mt2+1 -> v_sb[b]."""
                    ps = pool.tile([P, HW], F32, tag=pool_tag[id(pool)],
                                   name=f"v{b}_{mt2}")
                    for half in range(2):
                        mt = 2 * mt2 + half
                        for kt in range(KT):
                            nc.tensor.matmul(
                                ps[:, half * 512 : (half + 1) * 512],
                                lhsT=x_r[b][:, kt, mt * P : (mt + 1) * P],
                                rhs=wv_sb[:, kt, :],
                                start=(kt == 0),
                                stop=(kt == KT - 1) if not with_bias else False,
                            )
                        if with_bias:
                            nc.tensor.matmul(
                                ps[:, half * 512 : (half + 1) * 512],
                                lhsT=ones_sb, rhs=bv_sb,
                                start=False, stop=True,
                            )
                    nc.vector.tensor_copy(
                        out=v_sb[b][:, 2 * mt2 : 2 * mt2 + 2, :, 0:DK],
                        in_=ps.rearrange("p (k h t) -> p k h t", k=2, h=NH),
                    )

                def emit_proj_mms(b, at, ps, scs):
                    for sc in scs:
                        for kt in range(KT):
                            nc.tensor.matmul(
                                ps[:, sc * 512 : (sc + 1) * 512],
                                lhsT=wproj_r[:, kt, at * P : (at + 1) * P],
                                rhs=oT[b][:, kt, sc * 512 : (sc + 1) * 512],
                                start=(kt == 0),
                                stop=(kt == KT - 1),
                            )

                def emit_chain_proj(b, at, pool=None, ps=None):
                    """proj output-channel tile at -> DRAM (with residual)."""
                    if ps is None:
                        pool = pool if pool is not None else psQ
                        ps = pool.tile([P, HW], F32, tag=pool_tag[id(pool)],
                                       name=f"pj{b}_{at}")
                        emit_proj_mms(b, at, ps, range(SC))
                    yt = ytp.tile([P, HW], BF16, tag="y", name=f"yt{b}_{at}")
                    if with_bias:
                        nc.vector.scalar_tensor_tensor(
                            out=yt, in0=ps, scalar=bp_sb[:, at : at + 1],
                            in1=x_r[b][:, at, :],
                            op0=mybir.AluOpType.add, op1=mybir.AluOpType.add,
                        )
                    else:
                        nc.vector.tensor_tensor(
                            out=yt, in0=ps, in1=x_r[b][:, at, :],
                            op=mybir.AluOpType.add,
                        )
                    if b == 1:
                        # tail: split by partition half across both queues
                        nc.sync.dma_start(
                            out=out_d[b, at * P : at * P + DK, :],
                            in_=yt[0:DK, :],
                        )
                        nc.scalar.dma_start(
                            out=out_d[b, at * P + DK : (at + 1) * P, :],
                            in_=yt[DK:P, :],
                        )
                    else:
                        deng = nc.sync if at % 2 == 0 else nc.scalar
                        deng.dma_start(
                            out=out_d[b, at * P : (at + 1) * P, :], in_=yt
                        )

                def emit_pv_pass(b_p, a_p, pts_p, p_sc, jj):
                    for m in range(2):
                        nc.tensor.matmul(
                            pv_cur[p_sc % 2][m],
                            lhsT=v_sb[b_p][:, jj, 2 * a_p + m, :],
                            rhs=pts_p[m][:, jj, p_sc * 512 : (p_sc + 1) * 512],
                            start=(jj == 0),
                            stop=(jj == ST - 1),
                        )

                def emit_pv_alloc(b_p, a_p, p_sc):
                    pv_cur[p_sc % 2] = [
                        psV.tile([DK + 1, 512], F32, tag="pv",
                                 name=f"pv{b_p}_{a_p}_{p_sc}_{m}")
                        for m in range(2)
                    ]

                def emit_norm(b_p, a_p, p_sc):
                    for m in range(2):
                        pv = pv_cur[p_sc % 2][m]
                        rs = rcp.tile([1, 512], F32, tag="rs", name="rs")
                        nc.vector.tensor_copy(out=rs, in_=pv[DK : DK + 1, :])
                        rc = rcp.tile([1, 512], F32, tag="rc", name="rc")
                        nc.vector.reciprocal_approx_fast(out=rc, in_=rs)
                        rcb = rcbp.tile([DK, 512], F32, tag="rcb", name="rcb")
                        nc.gpsimd.partition_broadcast(rcb, rc)
                        nc.vector.tensor_tensor(
                            out=oT[b_p][m * DK : (m + 1) * DK, a_p,
                                      p_sc * 512 : (p_sc + 1) * 512],
                            in0=pv[0:DK, :], in1=rcb,
                            op=mybir.AluOpType.mult,
                        )

                def emit_attention(b, hooks, carry=None):
                    """Pair loop; hooks[p] emitted after pair p's jt loop.
                    Pair a's jt loop carries the previous pair's P@V (two sc
                    passes) — including the last pair of the previous elem
                    via `carry` = (b_prev, a_prev, pts_prev). Returns its own
                    last pair as the next carry."""
                    prev = carry
                    for a in range(NPAIR):
                        chain_q = list(hooks.get(a, []))
                        pts = [
                            ptp.tile([P, ST, HW], BF16, tag="pt",
                                     name=f"pt{b}_{a}_{m}", bufs=4)
                            for m in range(2)
                        ]
                        for jt in range(ST):
                            p_sc = 0 if jt < 4 else 1
                            if prev is not None and jt % 4 == 0:
                                emit_pv_alloc(prev[0], prev[1], p_sc)
                            pss = [
                                psS.tile([P, HW], F32, tag="sc",
                                         name=f"s{b}_{a}_{jt}_{m}")
                                for m in range(2)
                            ]
                            for sc in range(SC):
                                for m in range(2):
                                    nc.tensor.matmul(
                                        pss[m][:, sc * 512 : (sc + 1) * 512],
                                        lhsT=kT[b][:, a,
                                                   jt * P : (jt + 1) * P],
                                        rhs=qzp[b][:, a, m,
                                                   sc * 512 : (sc + 1) * 512],
                                        start=True,
                                        stop=True,
                                    )
                            if prev is not None:
                                for half in range(2):
                                    emit_pv_pass(prev[0], prev[1], prev[2],
                                                 p_sc, 2 * (jt % 4) + half)
                            for m in range(2):
                                nc.scalar.activation(
                                    out=pts[m][:, jt, :],
                                    in_=pss[m],
                                    func=mybir.ActivationFunctionType.Exp,
                                    scale=SCALE,
                                )
                            if prev is not None and jt % 4 == 3:
                                emit_norm(prev[0], prev[1], p_sc)
                            if jt in (0, 2, 4) and chain_q:
                                chain_q.pop(0)()
                        prev = (b, a, pts)
                    return prev

                def emit_pv_drain(prev):
                    for p_sc in range(SC):
                        emit_pv_alloc(prev[0], prev[1], p_sc)
                        for jj in range(ST):
                            emit_pv_pass(prev[0], prev[1], prev[2], p_sc, jj)
                        emit_norm(prev[0], prev[1], p_sc)

                pool_tag = {id(psS): "sc", id(psQ): "q"}
                pv_cur = [None, None]

                # ---------------- emission schedule ----------------
                # x(0) DMA (sync queue) runs concurrent with weight staging
                # (scalar/gpsimd queues); the barrier funnels all of it.
                emit_load(0)
                with tc.tile_pool(name="wpstage", bufs=2) as wpstagep:
                    for kt in range(KT):
                        wpst = wpstagep.tile([P, C], F32, tag="wp",
                                             name=f"wpst{kt}")
                        nc.gpsimd.dma_start(
                            out=wpst, in_=wproj_d[kt * P : (kt + 1) * P, :]
                        )
                        nc.gpsimd.tensor_copy(out=wproj_r[:, kt, :], in_=wpst)
                alloc_attn_bufs(0)
                # preamble: only q/k for pairs 0,1 (pair-0 scores gate on
                # them; keep the psS FIFO short). v(0) chains ride pair-0's
                # hook slots — P@V first touches them during pair 1.
                for i, (a, qk, pool) in enumerate(
                    [(0, 0, psQ), (0, 1, psS), (1, 0, psS), (1, 1, psQ)]
                ):
                    emit_chain_qk(0, a, qk, pool)
                emit_load(1)
                alloc_attn_bufs(1)

                # attention(0): finish qkv(0) pairs 2-3 early, then qkv(1)
                hooks0 = {
                    0: [lambda: emit_chain_v(0, 0, psQ),
                        lambda: emit_chain_v(0, 1, psQ),
                        lambda: emit_chain_v(0, 2, psQ)],
                    1: [lambda: emit_chain_v(0, 3, psQ),
                        lambda: emit_chain_qk(0, 2, 0, psQ),
                        lambda: emit_chain_qk(0, 2, 1, psQ)],
                    2: [lambda: emit_chain_qk(0, 3, 0, psQ),
                        lambda: emit_chain_qk(0, 3, 1, psQ),
                        lambda: emit_chain_v(1, 0, psQ)],
                    3: [lambda: emit_chain_v(1, 1, psQ),
                        lambda: emit_chain_qk(1, 0, 0, psQ),
                        lambda: emit_chain_qk(1, 0, 1, psQ)],
                }
                carry = emit_attention(0, hooks0)

                # attention(1): remaining qkv(1) chains (>=1 pair lookahead),
                # then proj(0); elem0's pair-3 P@V rides in via `carry`.
                hooks1 = {
                    0: [lambda: emit_chain_qk(1, 1, 0, psQ),
                        lambda: emit_chain_qk(1, 1, 1, psQ),
                        lambda: emit_chain_v(1, 2, psQ)],
                    1: [lambda: emit_chain_v(1, 3, psQ),
                        lambda: emit_chain_qk(1, 2, 0, psQ),
                        lambda: emit_chain_qk(1, 2, 1, psQ)],
                    2: [lambda: emit_chain_qk(1, 3, 0, psQ),
                        lambda: emit_chain_qk(1, 3, 1, psQ),
                        lambda: emit_chain_proj(0, 0)],
                    3: [lambda: emit_chain_proj(0, 1),
                        lambda: emit_chain_proj(0, 2),
                        lambda: emit_chain_proj(0, 3)],
                }
                carry = emit_attention(1, hooks1, carry)

                # drain elem1's last pair with proj(1) interleaved: the sc0
                # proj matmuls run while the pass-1 norm's DVE chain drains.
                b_p, a_p, pts_p = carry
                emit_pv_alloc(b_p, a_p, 0)
                for jj in range(ST):
                    emit_pv_pass(b_p, a_p, pts_p, 0, jj)
                emit_norm(b_p, a_p, 0)
                emit_pv_alloc(b_p, a_p, 1)
                for jj in range(ST):
                    emit_pv_pass(b_p, a_p, pts_p, 1, jj)
                lanes = [psQ, psS, psS]
                pjs = []
                for at in range(3):
                    ps = lanes[at].tile([P, HW], F32,
                                        tag=pool_tag[id(lanes[at])],
                                        name=f"pj1_{at}")
                    emit_proj_mms(1, at, ps, [0])
                    pjs.append(ps)
                emit_norm(b_p, a_p, 1)
                for at in range(3):
                    emit_proj_mms(1, at, pjs[at], [1])
                    emit_chain_proj(1, at, ps=pjs[at])
                emit_chain_proj(1, 3, psQ)

    nc.finalize()
    return nc


_CACHE = {}


def _get_program(with_bias: bool, mm_dtype: str = MM_DTYPE):
    key = (with_bias,)
    if key not in _CACHE:
        _CACHE[key] = build_program(with_bias, mm_dtype)
    return _CACHE[key]


def kernel(x, w_qkv, b_qkv, w_proj, b_proj):
    x = np.ascontiguousarray(np.asarray(x, dtype=np.float32)).reshape(B, C, HW)
    w_qkv = np.ascontiguousarray(np.asarray(w_qkv, dtype=np.float32))
    b_qkv = np.ascontiguousarray(np.asarray(b_qkv, dtype=np.float32))
    w_proj = np.ascontiguousarray(np.asarray(w_proj, dtype=np.float32))
    b_proj = np.ascontiguousarray(np.asarray(b_proj, dtype=np.float32))

    with_bias = bool(np.any(b_qkv) or np.any(b_proj))
    nc = _get_program(with_bias, MM_DTYPE)

    in_maps = []
    for i in range(NCORES):
        m = {
            "x": x[i * BPC : (i + 1) * BPC],
            "w_qkv": w_qkv,
            "w_proj": w_proj,
        }
        if with_bias:
            m["b_qkv"] = b_qkv
            m["b_proj"] = b_proj
        in_maps.append(m)

    from concourse.bass_utils import run_bass_kernel_spmd

    res = run_bass_kernel_spmd(nc, in_maps, core_ids=list(range(NCORES)))
    out = np.concatenate(
        [np.asarray(r["out"], dtype=np.float32) for r in res.results], axis=0
    )
    return out.reshape(B, C, 32, 32)



# revision 33
# speedup vs baseline: 1.2744x; 1.2744x over previous
"""Trainium2 Bass kernel for nn_AttentionBlock (B=16, C=512, H=W=32, 8 heads).

Sharding: data-parallel over batch across 8 NeuronCores (2 batch elems/core).
No collectives: each core runs the same NEFF on its own batch slice.

v5 — fp8 DoubleRow + host-prepped inputs + pipeline shaping (~188us,
vs 240us bf16 baseline):
  - QKV, P@V and proj matmuls run fp8e4 with MatmulPerfMode.DoubleRow:
    each instruction contracts TWO 128-row k-tiles (K=256) at the same
    ~215ns as a bf16 K=128 matmul -> 2x fewer PE instructions there.
    (HW-probed: DR does NOT double column rate; it doubles contraction.
    DR weights need M % 32 == 0.)
  - Scores are plain fp8 (contraction is only d_k=64, zero-padded q
    halves to K=128; nothing for DR to pair).
  - ScalarE is the pacer: 128 exp ACTs x ~1.04us = ~134us busy (70% of
    span). exp(s*scale - 2.5) writes pts as fp8e4 directly; the shift
    keeps exp under fp8e4 max and softmax is shift-invariant.
  - Host-side prep (_prep_inputs): weights/x are fp8-quantized and
    pre-packed into the exact SBUF layouts (identical rounding to the
    on-chip DVE cast), so startup ships 1.25MB instead of 5MB and no
    on-chip weight rearranges/x casts exist. Residual x stays f32 and
    loads lazily (first read at proj time). Per-ring DMA is only
    ~150GB/s, so startup-critical bytes are split across the scalar/
    sync/gpsimd rings, wqk8 pair-major so pair 0 lands first.
  - v8 has 32 ones-columns (M=96); psum rows 64..95 hold the softmax
    denominator. The denominator row is copied to SBUF before
    reciprocal_approx_fast (the custom DVE op reads garbage from PSUM
    on HW), then GpSimd partition-broadcast + DVE multiply normalize.
    (HW: DVE tensor_tensor divide fails the ISA check; GpSimd cannot
    read PSUM at all.)
  - Chains/proj are emitted in sc-HALVES ([128,512] psum tiles, psQ
    bufs=2), one popped per jt 0..5 of each pair loop and flushed after
    the NEXT jt's score matmuls so a psQ-stalled chain cannot starve
    the ACT stream.
  - Drain: PV1 runs in the freed psQ banks right after the last ACT in
    parallel with norm0 (psV); norms are phase-interleaved with rs
    copies on the now-idle ScalarE; proj runs in the freed psS banks,
    sc0 gated only on norm0. Dummy matmuls at startup and in the drain
    keep the PE p-state ramped (cold PE is 2-3x slower per matmul).
  PSUM budget: scores 2x[128,1024] (4 banks) + PV 2x[96,512] (2 banks)
  + chain/proj halves 2x[128,512] (2 banks) = 8 banks (fully committed;
  this is what pins the remaining ~15us of ACT-stream gaps).
"""

import numpy as np

import concourse.bacc as bacc
import concourse.bass as bass
import concourse.mybir as mybir
import concourse.tile as tile

F32 = mybir.dt.float32
BF16 = mybir.dt.bfloat16
FP8 = mybir.dt.float8e4
DRM = mybir.MatmulPerfMode.DoubleRow

B, C, HW, NH, DK = 16, 512, 1024, 8, 64
NCORES = 8
BPC = B // NCORES          # batch elems per core
P = 128
KT = C // P                # 4 contraction tiles over C
KTP = KT // 2              # 2 DoubleRow k-tile pairs
NPAIR = NH // 2            # 4 head pairs
SC = HW // 512             # 2 s-chunks of 512
ST = HW // P               # 8 s-tiles of 128 (j tiles)
JP = ST // 2               # 4 DoubleRow j-tile pairs for P@V
MV = DK + 32               # 96: v cols + 32 ones cols (DR needs M%32==0)
SCALE = DK ** -0.5
SHIFT = 2.5  # exp(s*SCALE - SHIFT): keeps exp under fp8e4 max (448);
             # softmax is shift-invariant so the result is unchanged

MM_DTYPE = "fp8dr"  # kept for test.py compat


def build_program(with_bias: bool, mm_dtype: str = MM_DTYPE):
    nc = bacc.Bacc(None, target_bir_lowering=False, debug=False)

    # Host-prepped: x8t/xt are [BPC, P, KT*HW] (partition-major, so DMAs
    # are contiguous per partition line); weights are pre-packed into the
    # exact SBUF layouts (see _prep_inputs).
    x8t_d = nc.dram_tensor("x8t", [BPC, P, KT * HW], FP8,
                           kind="ExternalInput")
    xt_d = nc.dram_tensor("xt", [BPC, P, KT, HW], F32, kind="ExternalInput")
    wqk8_d = nc.dram_tensor("wqk8", [P, KT * NPAIR * 2 * P], FP8,
                            kind="ExternalInput")
    wv8_d = nc.dram_tensor("wv8", [P, KT * C], FP8, kind="ExternalInput")
    wp8_d = nc.dram_tensor("wp8", [P, KT * C], FP8, kind="ExternalInput")
    if with_bias:
        bqkv_d = nc.dram_tensor("b_qkv", [3 * C], F32, kind="ExternalInput")
        bproj_d = nc.dram_tensor("b_proj", [C], F32, kind="ExternalInput")
    out_d = nc.dram_tensor("out", [BPC, C, HW], BF16,
                           kind="ExternalOutput")

    with tile.TileContext(nc) as tc:
        with tc.tile_pool(name="consts", bufs=1) as consts:
            # Pair-packed q/k weights, fp8, kt-major so a DoubleRow lhsT
            # slice [:, 2t:2t+2, a, qk, :] pairs two kt tiles:
            # cols of (pair a, qk) are [w_{2a} | w_{2a+1}] head halves.
            wqk8 = consts.tile([P, NPAIR, 2, KT, P], FP8)
            wv8 = consts.tile([P, KT, C], FP8)
            wp8 = consts.tile([P, KT, C], FP8)
            warm_i = consts.tile([1, DK], F32)
            warm_o = consts.tile([1, DK], F32)
            # Zero-padded q (fp8): per (pair, m) the scores rhs is
            # [128, S] with q_h on partition half m and ZEROS on the other
            # half, so score matmuls run K=128 (no tiling-mode switch).
            qzp = [consts.tile([P, NPAIR, 2, HW], FP8, name=f"qzp{b}")
                   for b in range(BPC)]
            if with_bias:
                bq_sb = consts.tile([P, NPAIR], F32)
                bk_sb = consts.tile([P, NPAIR], F32)
                bv_sb = consts.tile([1, C], FP8)
                bp_sb = consts.tile([P, KT], F32)
                ones_f32 = consts.tile([1, P], F32)
                nc.vector.memset(ones_f32, 1.0)
                ones8 = consts.tile([1, P], FP8)
                nc.vector.tensor_copy(out=ones8, in_=ones_f32)

            nc.vector.memset(warm_i, 1.0)
            dum8 = consts.tile([P, 512], FP8)
            nc.vector.memset(dum8, 0.0)
            nbias = consts.tile([P, 1], F32)
            nc.vector.memset(nbias, -SHIFT)
            # half-masks: qzp is written as q * mask so its zero half never
            # needs a bulk memset
            hmask = consts.tile([P, 2], F32)
            nc.vector.memset(hmask, 0.0)
            nc.vector.memset(hmask[0:DK, 0:1], 1.0)
            nc.vector.memset(hmask[DK:P, 1:2], 1.0)

            with (
                tc.tile_pool(name="xf", bufs=1) as xfp,
                tc.tile_pool(name="x8", bufs=2) as x8p,
                tc.tile_pool(name="qk", bufs=2) as qkp,
                tc.tile_pool(name="vp", bufs=2) as vpp,
                tc.tile_pool(name="pt", bufs=5) as ptp,
                tc.tile_pool(name="ot", bufs=2) as otp,
                tc.tile_pool(name="yt", bufs=4) as ytp,
                tc.tile_pool(name="rc", bufs=2) as rcp,
                tc.tile_pool(name="rcb", bufs=2) as rcbp,
                tc.tile_pool(name="psS", bufs=2, space="PSUM") as psS,
                tc.tile_pool(name="psQ", bufs=2, space="PSUM") as psQ,
                tc.tile_pool(name="psV", bufs=2, space="PSUM") as psV,
            ):
                xf = [[None, None], [None, None]]
                x8 = [None, None]
                kT = [None, None]
                v8 = [None, None]
                oT = [None, None]

                def emit_load(b):
                    """DMA the host-prepped fp8 x straight into SBUF,
                    split across the sync and gpsimd rings."""
                    x8[b] = x8p.tile([P, KT, HW], FP8, tag="x8",
                                     name=f"x8_{b}")
                    x8f = x8[b].rearrange("p k s -> p (k s)")
                    half = KT * HW // 2
                    nc.sync.dma_start(
                        out=x8f[:, 0:half], in_=x8t_d[b][:, 0:half]
                    )
                    nc.gpsimd.dma_start(
                        out=x8f[:, half:], in_=x8t_d[b][:, half:]
                    )

                def emit_load_res(b, ch, eng):
                    """DMA one f32 residual chunk (not startup-critical:
                    first read at proj time)."""
                    xc = xfp.tile([P, 2, HW], F32, tag=f"x{b}_{ch}",
                                  name=f"x{b}_{ch}")
                    eng.dma_start(
                        out=xc, in_=xt_d[b][:, 2 * ch : 2 * ch + 2, :]
                    )
                    xf[b][ch] = xc

                def alloc_attn_bufs(b):
                    kT[b] = qkp.tile([P, NPAIR, HW], FP8, tag="kT",
                                     name=f"kT{b}")
                    v8[b] = vpp.tile([P, ST, NH, MV], FP8, tag="v",
                                     name=f"v{b}")
                    oT[b] = otp.tile([P, KT, HW], FP8, tag="oT",
                                     name=f"oT{b}")
                    nc.gpsimd.memset(
                        v8[b].rearrange("p st h t -> p (st h) t")[:, :, DK:],
                        1.0,
                    )

                def chain_qk_half(b, a, qk, sc):
                    """Half a q-or-k chain: one 512-col chunk, 2 DR matmuls,
                    then its qzp/kT write."""
                    ps = psQ.tile([P, 512], F32, tag="q",
                                  name=f"qk{b}_{a}_{qk}_{sc}")
                    lo, hi = sc * 512, (sc + 1) * 512
                    for t in range(KTP):
                        nc.tensor.matmul(
                            ps,
                            lhsT=wqk8[:, a, qk, 2 * t : 2 * t + 2, :],
                            rhs=x8[b][:, 2 * t : 2 * t + 2, lo:hi],
                            start=(t == 0),
                            stop=(t == KTP - 1),
                            perf_mode=DRM,
                        )
                    if qk == 0:
                        for m in range(2):
                            if with_bias:
                                nc.vector.tensor_scalar(
                                    out=qzp[b][:, a, m, lo:hi], in0=ps,
                                    scalar1=bq_sb[:, a : a + 1],
                                    scalar2=hmask[:, m : m + 1],
                                    op0=mybir.AluOpType.add,
                                    op1=mybir.AluOpType.mult,
                                )
                            else:
                                nc.vector.tensor_scalar(
                                    out=qzp[b][:, a, m, lo:hi], in0=ps,
                                    scalar1=hmask[:, m : m + 1],
                                    scalar2=None,
                                    op0=mybir.AluOpType.mult,
                                )
                    elif with_bias:
                        nc.vector.tensor_scalar(
                            out=kT[b][:, a, lo:hi], in0=ps,
                            scalar1=bk_sb[:, a : a + 1], scalar2=None,
                            op0=mybir.AluOpType.add,
                        )
                    else:
                        nc.vector.tensor_copy(out=kT[b][:, a, lo:hi], in_=ps)

                def chain_v_half(b, mt):
                    """v rows for s-tile mt -> v8[b] (fp8)."""
                    ps = psQ.tile([P, 512], F32, tag="q", name=f"v{b}_{mt}")
                    for t in range(KTP):
                        nc.tensor.matmul(
                            ps,
                            lhsT=x8[b][:, 2 * t : 2 * t + 2,
                                       mt * P : (mt + 1) * P],
                            rhs=wv8[:, 2 * t : 2 * t + 2, :],
                            start=(t == 0),
                            stop=(t == KTP - 1) if not with_bias else False,
                            perf_mode=DRM,
                        )
                    if with_bias:
                        nc.tensor.matmul(
                            ps, lhsT=ones8, rhs=bv_sb,
                            start=False, stop=True,
                        )
                    nc.vector.tensor_copy(
                        out=v8[b][:, mt, :, 0:DK],
                        in_=ps.rearrange("p (h t) -> p h t", h=NH),
                    )

                def emit_dummies(n, cols=512):
                    """p-state keepalive: matmuls on a zero const tile into
                    dead psum (no readers) so the PE clock stays ramped."""
                    for _ in range(n):
                        ps = psQ.tile([P, 512], F32, tag="q", name="dum")
                        nc.tensor.matmul(
                            ps[:, 0:cols], lhsT=dum8[:, 0:P],
                            rhs=dum8[:, 0:cols],
                            start=True, stop=True,
                        )

                def proj_half(b, at, sc, pool=None):
                    """proj chunk (at, sc): 2 DR matmuls + residual add; DMA
                    the at-tile after its second half."""
                    pool = pool if pool is not None else psQ
                    ps = pool.tile([P, 512], F32, tag=pool_tag2[id(pool)],
                                   name=f"pj{b}_{at}_{sc}")
                    lo, hi = sc * 512, (sc + 1) * 512
                    for t in range(KTP):
                        nc.tensor.matmul(
                            ps,
                            lhsT=wp8[:, 2 * t : 2 * t + 2,
                                     at * P : (at + 1) * P],
                            rhs=oT[b][:, 2 * t : 2 * t + 2, lo:hi],
                            start=(t == 0),
                            stop=(t == KTP - 1),
                            perf_mode=DRM,
                        )
                    if sc == 0:
                        yts[(b, at)] = ytp.tile([P, HW], BF16, tag="y",
                                                name=f"yt{b}_{at}")
                    yt = yts[(b, at)]
                    xres = xf[b][at // 2][:, at % 2, lo:hi]
                    if with_bias:
                        nc.vector.scalar_tensor_tensor(
                            out=yt[:, lo:hi], in0=ps,
                            scalar=bp_sb[:, at : at + 1], in1=xres,
                            op0=mybir.AluOpType.add, op1=mybir.AluOpType.add,
                        )
                    else:
                        nc.vector.tensor_tensor(
                            out=yt[:, lo:hi], in0=ps, in1=xres,
                            op=mybir.AluOpType.add,
                        )
                    if sc == SC - 1:
                        if b == 1:
                            # tail: split by partition half across both
                            # queues (ScalarE has finished all exps by now)
                            nc.sync.dma_start(
                                out=out_d[b, at * P : at * P + DK, :],
                                in_=yt[0:DK, :],
                            )
                            nc.scalar.dma_start(
                                out=out_d[b, at * P + DK : (at + 1) * P, :],
                                in_=yt[DK:P, :],
                            )
                        else:
                            nc.sync.dma_start(
                                out=out_d[b, at * P : (at + 1) * P, :],
                                in_=yt,
                            )

                def emit_pv_pass(b_p, a_p, pts_p, p_sc, jp):
                    """One DR P@V unit: j-tiles 2jp, 2jp+1 for both heads."""
                    for m in range(2):
                        nc.tensor.matmul(
                            pv_cur[p_sc % 2][m],
                            lhsT=v8[b_p][:, 2 * jp : 2 * jp + 2,
                                         2 * a_p + m, :],
                            rhs=pts_p[m][:, 2 * jp : 2 * jp + 2,
                                         p_sc * 512 : (p_sc + 1) * 512],
                            start=(jp == 0),
                            stop=(jp == JP - 1),
                            perf_mode=DRM,
                        )

                def emit_pv_alloc(b_p, a_p, p_sc):
                    pv_cur[p_sc % 2] = [
                        psV.tile([MV, 512], F32, tag="pv",
                                 name=f"pv{b_p}_{a_p}_{p_sc}_{m}")
                        for m in range(2)
                    ]

                def emit_norm(b_p, a_p, p_sc, tiles=None, rs_eng=None):
                    """Normalize one P@V pass. Emission is phase-interleaved
                    (both rs+recips, both bcasts, both mults) so the two
                    m-units pipeline across DVE/GpSimd instead of
                    serializing behind the first mult."""
                    tiles = tiles if tiles is not None else pv_cur[p_sc % 2]
                    rcs = []
                    for m in range(2):
                        pv = tiles[m]
                        rs = rcp.tile([1, 512], F32, tag="rs", name="rs")
                        if rs_eng is None:
                            nc.vector.tensor_copy(
                                out=rs, in_=pv[DK : DK + 1, :])
                        else:
                            rs_eng.copy(out=rs, in_=pv[DK : DK + 1, :])
                        rc = rcp.tile([1, 512], F32, tag="rc", name="rc")
                        nc.vector.reciprocal_approx_fast(out=rc, in_=rs)
                        rcs.append(rc)
                    rcbs = []
                    for m in range(2):
                        rcb = rcbp.tile([DK, 512], F32, tag="rcb", name="rcb")
                        nc.gpsimd.partition_broadcast(rcb, rcs[m])
                        rcbs.append(rcb)
                    for m in range(2):
                        nc.vector.tensor_tensor(
                            out=oT[b_p][m * DK : (m + 1) * DK, a_p,
                                      p_sc * 512 : (p_sc + 1) * 512],
                            in0=tiles[m][0:DK, :], in1=rcbs[m],
                            op=mybir.AluOpType.mult,
                        )

                def emit_attention(b, hooks, carry=None, self_pv_last=False):
                    """Pair loop; hooks[p] is a list of HALF-chain closures,
                    one popped per jt in 0..5. Pair a's jt loop carries the
                    previous pair's P@V (two sc passes) — including the last
                    pair of the previous elem via `carry` = (b_prev, a_prev,
                    pts_prev). Returns its own last pair as the next carry.
                    With self_pv_last, the final pair also runs its own
                    p_sc=0 P@V at jts 3/5/7 (jp lags the ACT stream by 2 jts
                    so the PE FIFO never blocks on an in-flight ACT)."""
                    prev = carry
                    deferred = [None]
                    for a in range(NPAIR):
                        chain_q = list(hooks.get(a, []))
                        last = self_pv_last and a == NPAIR - 1
                        pts = [
                            ptp.tile([P, ST, HW], FP8, tag="pt",
                                     name=f"pt{b}_{a}_{m}", bufs=5)
                            for m in range(2)
                        ]
                        for jt in range(ST):
                            p_sc = 0 if jt < 4 else 1
                            if prev is not None and jt % 4 == 0:
                                emit_pv_alloc(prev[0], prev[1], p_sc)
                            pss = [
                                psS.tile([P, HW], F32, tag="sc",
                                         name=f"s{b}_{a}_{jt}_{m}")
                                for m in range(2)
                            ]
                            for sc in range(SC):
                                for m in range(2):
                                    nc.tensor.matmul(
                                        pss[m][:, sc * 512 : (sc + 1) * 512],
                                        lhsT=kT[b][:, a,
                                                   jt * P : (jt + 1) * P],
                                        rhs=qzp[b][:, a, m,
                                                   sc * 512 : (sc + 1) * 512],
                                        start=True,
                                        stop=True,
                                    )
                            # flush the previous jt's chain AFTER this jt's
                            # scores: a psQ-stalled chain then can't delay
                            # the ACT stream's inputs in the PE FIFO.
                            if deferred[0] is not None:
                                deferred[0]()
                                deferred[0] = None
                            if prev is not None:
                                emit_pv_pass(prev[0], prev[1], prev[2],
                                             p_sc, jt % 4)
                            if last and jt in (3, 5, 7):
                                if jt == 3:
                                    emit_pv_alloc(b, a, 0)
                                emit_pv_pass(b, a, pts, 0, (jt - 3) // 2)
                            for m in range(2):
                                nc.scalar.activation(
                                    out=pts[m][:, jt, :],
                                    in_=pss[m],
                                    func=mybir.ActivationFunctionType.Exp,
                                    scale=SCALE, bias=nbias[:, 0:1],
                                )
                            if prev is not None and jt % 4 == 3:
                                emit_norm(prev[0], prev[1], p_sc)
                            if jt < 6 and chain_q:
                                deferred[0] = chain_q.pop(0)
                        prev = (b, a, pts)
                    return prev

                pv_cur = [None, None]
                yts = {}
                pool_tag2 = {id(psQ): "q", id(psS): "sc"}

                def qk_halves(b, a, qk):
                    return [lambda sc=sc: chain_qk_half(b, a, qk, sc)
                            for sc in range(SC)]

                def v_halves(b, mt2):
                    return [lambda mt=mt: chain_v_half(b, mt)
                            for mt in (2 * mt2, 2 * mt2 + 1)]

                def proj_halves(b, at):
                    return [lambda sc=sc: proj_half(b, at, sc)
                            for sc in range(SC)]

                # ---------------- emission schedule ----------------
                emit_dummies(12)
                # x(0) DMA + fp8 cast first; wqkv DMAs all issued up front
                # on the scalar HWDGE ring (ScalarE is idle until the first
                # scores land); wproj rides the slow gpsimd SWDGE.
                # weights: direct DMA into the pre-packed SBUF layouts.
                # wqk8 (pair-0 chains gate on it) on the scalar ring; wv8
                # and wp8 on sync/gpsimd behind the x8 halves.
                wqk8f = wqk8.rearrange("p a b c d -> p (a b c d)")
                csz = 2 * KT * P
                for a in range(NPAIR):
                    nc.scalar.dma_start(
                        out=wqk8f[:, a * csz : (a + 1) * csz],
                        in_=wqk8_d[:, a * csz : (a + 1) * csz],
                    )
                # Warm the ScalarE exp table set (after the DMA trigger so
                # it doesn't delay it; before any real ACT).
                nc.scalar.activation(
                    out=warm_o, in_=warm_i,
                    func=mybir.ActivationFunctionType.Exp, scale=1.0,
                )
                emit_load(0)
                nc.sync.dma_start(
                    out=wv8.rearrange("p a b -> p (a b)"), in_=wv8_d[:, :]
                )
                nc.gpsimd.dma_start(
                    out=wp8.rearrange("p a b -> p (a b)"), in_=wp8_d[:, :]
                )
                if with_bias:
                    with tc.tile_pool(name="bstage", bufs=1) as bstagep:
                        b3 = bqkv_d[:].rearrange("(h t) -> h t", t=3 * DK)
                        for m in range(2):
                            nc.sync.dma_start(
                                out=bq_sb[m * DK : (m + 1) * DK, :],
                                in_=b3[m::2, 0:DK].rearrange("a t -> t a"),
                            )
                            nc.sync.dma_start(
                                out=bk_sb[m * DK : (m + 1) * DK, :],
                                in_=b3[m::2, DK : 2 * DK].rearrange(
                                    "a t -> t a"),
                            )
                        bv_st = bstagep.tile([1, C], F32, tag="bv")
                        nc.sync.dma_start(
                            out=bv_st,
                            in_=b3[:, 2 * DK :].rearrange(
                                "h t -> (h t)")[None, :],
                        )
                        nc.vector.tensor_copy(out=bv_sb, in_=bv_st)
                        nc.sync.dma_start(
                            out=bp_sb,
                            in_=bproj_d[:].rearrange("(a p) -> p a", p=P),
                        )

                alloc_attn_bufs(0)
                # preamble: only q/k for pairs 0,1 (pair-0 scores gate on
                # them; keep the FIFO short). v(0) chains ride pair-0's
                # hook slots — P@V first touches them during pair 1.
                for a, qk in [(0, 0), (0, 1), (1, 0), (1, 1)]:
                    for h in qk_halves(0, a, qk):
                        h()
                emit_load(1)
                alloc_attn_bufs(1)
                # residual f32 x rides the rings behind everything
                # startup-critical; first read at proj(0)/proj(1).
                emit_load_res(0, 0, nc.sync)
                emit_load_res(0, 1, nc.gpsimd)
                emit_load_res(1, 0, nc.sync)
                emit_load_res(1, 1, nc.gpsimd)

                # attention(0): finish qkv(0) pairs 2-3 early, then qkv(1)
                hooks0 = {
                    0: v_halves(0, 0) + v_halves(0, 1) + v_halves(0, 2),
                    1: v_halves(0, 3) + qk_halves(0, 2, 0)
                       + qk_halves(0, 2, 1),
                    2: qk_halves(0, 3, 0) + qk_halves(0, 3, 1)
                       + v_halves(1, 0),
                    3: v_halves(1, 1) + qk_halves(1, 0, 0)
                       + qk_halves(1, 0, 1),
                }
                carry = emit_attention(0, hooks0)

                # attention(1): remaining qkv(1) chains (>=1 pair lookahead),
                # then proj(0); elem0's pair-3 P@V rides in via `carry`.
                hooks1 = {
                    0: qk_halves(1, 1, 0) + qk_halves(1, 1, 1)
                       + v_halves(1, 2),
                    1: v_halves(1, 3) + qk_halves(1, 2, 0)
                       + qk_halves(1, 2, 1),
                    2: qk_halves(1, 3, 0) + qk_halves(1, 3, 1)
                       + proj_halves(0, 0),
                    3: proj_halves(0, 1) + proj_halves(0, 2)
                       + proj_halves(0, 3),
                }
                carry = emit_attention(1, hooks1, carry)

                # drain elem1's last pair. PV1 goes into the freed psQ
                # banks so it starts right after the last ACT, in parallel
                # with norm0 (which reads the psV tiles); proj runs in the
                # freed psS banks, sc0 gated only on norm0, sc1 on norm1.
                b_p, a_p, pts_p = carry
                emit_pv_alloc(b_p, a_p, 0)
                for jp in range(JP):
                    emit_pv_pass(b_p, a_p, pts_p, 0, jp)
                pv_cur[1] = [
                    psQ.tile([MV, 512], F32, tag="q", name=f"pvq_{m}")
                    for m in range(2)
                ]
                for jp in range(JP):
                    emit_pv_pass(b_p, a_p, pts_p, 1, jp)
                emit_norm(b_p, a_p, 0, rs_eng=nc.scalar)
                emit_norm(b_p, a_p, 1, rs_eng=nc.scalar)
                for _ in range(6):
                    ps = psS.tile([P, 512], F32, tag="sc", name="dum")
                    nc.tensor.matmul(ps, lhsT=dum8[:, 0:P], rhs=dum8,
                                     start=True, stop=True)
                for at in range(4):
                    proj_half(1, at, 0, pool=psS)
                for at in range(4):
                    proj_half(1, at, 1, pool=psS)

    nc.finalize()
    return nc


_CACHE = {}


def _get_program(with_bias: bool, mm_dtype: str = MM_DTYPE):
    key = (with_bias,)
    if key not in _CACHE:
        _CACHE[key] = build_program(with_bias, mm_dtype)
    return _CACHE[key]


def _prep_inputs(x, w_qkv, b_qkv, w_proj, b_proj):
    """Host-side: fp8-quantize and pre-pack into the exact SBUF layouts
    (identical rounding to the on-chip DVE cast, ml_dtypes e4m3)."""
    import ml_dtypes

    FP8NP = ml_dtypes.float8_e4m3
    x = np.ascontiguousarray(np.asarray(x, dtype=np.float32)).reshape(
        B, C, HW)
    w_qkv = np.asarray(w_qkv, dtype=np.float32)
    w_proj = np.asarray(w_proj, dtype=np.float32)

    # [B, C, HW] -> [B, P, KT, HW]: partition p holds C-rows kt*128+p
    xt = np.ascontiguousarray(
        x.reshape(B, KT, P, HW).transpose(0, 2, 1, 3))
    x8t = np.ascontiguousarray(xt.astype(FP8NP)).reshape(B, P, KT * HW)

    w8 = w_qkv.astype(FP8NP)
    w3 = w8.reshape(KT, P, NH, 3, DK)  # [kt, p, h, {q,k,v}, d]
    # wqk8[p, a, qk, kt, m*DK+d] = w8[kt*P+p, (2a+m)*3*DK + qk*DK + d]
    # (pair-major so pair 0's weights can DMA first)
    t1 = w3[:, :, :, 0:2, :].reshape(KT, P, NPAIR, 2, 2, DK)
    wqk8 = np.ascontiguousarray(
        t1.transpose(1, 2, 4, 0, 3, 5)).reshape(P, NPAIR * 2 * KT * P)
    # wv8[p, kt, h*DK+d] = w8[kt*P+p, h*3*DK + 2*DK + d]
    wv8 = np.ascontiguousarray(
        w3[:, :, :, 2, :].reshape(KT, P, C).transpose(1, 0, 2)).reshape(
        P, KT * C)
    # wp8[p, kt, c] = wproj8[kt*P+p, c]
    wp8 = np.ascontiguousarray(
        w_proj.astype(FP8NP).reshape(KT, P, C).transpose(1, 0, 2)).reshape(
        P, KT * C)
    return x8t, xt, wqk8, wv8, wp8


def make_in_maps(inputs):
    """Build per-core input maps from the raw setup_inputs() dict."""
    b_qkv = np.asarray(inputs["b_qkv"], dtype=np.float32)
    b_proj = np.asarray(inputs["b_proj"], dtype=np.float32)
    with_bias = bool(np.any(b_qkv) or np.any(b_proj))
    x8t, xt, wqk8, wv8, wp8 = _prep_inputs(
        inputs["x"], inputs["w_qkv"], b_qkv, inputs["w_proj"], b_proj)
    in_maps = []
    for i in range(NCORES):
        m = {
            "x8t": x8t[i * BPC : (i + 1) * BPC],
            "xt": xt[i * BPC : (i + 1) * BPC],
            "wqk8": wqk8,
            "wv8": wv8,
            "wp8": wp8,
        }
        if with_bias:
            m["b_qkv"] = b_qkv
            m["b_proj"] = b_proj
        in_maps.append(m)
    return with_bias, in_maps


def kernel(x, w_qkv, b_qkv, w_proj, b_proj):
    with_bias, in_maps = make_in_maps({
        "x": x, "w_qkv": w_qkv, "b_qkv": b_qkv,
        "w_proj": w_proj, "b_proj": b_proj,
    })
    nc = _get_program(with_bias, MM_DTYPE)

    from concourse.bass_utils import run_bass_kernel_spmd

    res = run_bass_kernel_spmd(nc, in_maps, core_ids=list(range(NCORES)))
    out = np.concatenate(
        [np.asarray(r["out"], dtype=np.float32) for r in res.results], axis=0
    )
    return out.reshape(B, C, 32, 32)
